# revision 1
# baseline (speedup 1.0000x reference)
"""Trainium2 Bass kernel for CustomMultiheadAttention.

Problem shapes: query/key/value [2048, 4, 1024] f32, causal mask [2048, 2048],
Wq/Wk/Wv/Wo [1024, 1024] (torch Linear layout [out, in]), biases [1024].
16 heads, head dim 64.

Sharding over 8 cores: core c -> (batch b = c // 2, head-group hg = c % 2).
Each core computes 8 heads (an E-slice of 512 rows of Wq/Wk/Wv, 512 cols of
Wo) for one batch. Host sums the two partial output projections per batch and
adds bo.

Device algorithm per core (all matmul inputs bf16, f32 PSUM accumulation):
  1. Q^T[d,t] = Wq_slice^T.T-contraction over e of Xq^T;   + bq (per-partition)
     K^T[d,s] likewise; V[s,d] = Xv @ Wv_slice^T + bv (strided add, with a
     ones-column appended per head for the softmax denominator).
  2. Per t-chunk (512), per head: scores^T[s,t] tiles [128, 512] via K=64
     matmuls; exp(scores/8) on ACT (scores ~ N(0,1): no max-subtraction
     needed), with partial-span exp + memset + triangular boundary mask on
     diagonal tiles; PV matmuls (pipelined one group behind the scores so
     ACT latency is hidden) accumulate [65, 512] = [num^T; den].
     Normalize: recip = 1/den (DVE), gpsimd partition_broadcast across the
     64 d-partitions, multiply -> attnT bf16 [hd, t].
  3. Output projection out_part[t, f] = attnT.T @ Wo_slice^T, emitted as
     deferred "filler" tiles popped into TensorE gaps of the next t-chunk's
     attention stream (keeps the PE busy so the HAM clock-gate stays warm).
"""

import math
import os
import sys

import numpy as np

for _p in ("/opt/trn_rl_repo", os.path.expanduser("~/.axon_site/_ro/trn_rl_repo")):
    if os.path.isdir(_p) and _p not in sys.path:
        sys.path.insert(0, _p)

import ml_dtypes  # noqa: E402

import concourse.bass as bass  # noqa: E402
import concourse.tile as tile  # noqa: E402
from concourse import bacc, bass_utils, library_config, mybir  # noqa: E402

# Problem constants
T, S, B, E, H = 2048, 2048, 4, 1024, 16
D = E // H  # 64
NCORES = 8
HC = H // 2  # heads per core
EH = HC * D  # 512 per-core E-slice
P = 128
TC = 512  # t-chunk
NT = T // TC  # 4
NSB = S // P  # 16 s-blocks
KO = E // P  # 8 contraction chunks for projections
KHD = EH // P  # 4 contraction chunks for out proj
VW = D + 1  # 65: head V width incl ones column
BF16 = mybir.dt.bfloat16
F32 = mybir.dt.float32
NPBF16 = ml_dtypes.bfloat16

_CACHE: dict = {}


def _build_nc():
    nc = bacc.Bacc(
        "TRN2",
        target_bir_lowering=False,
        debug=False,
        enable_asserts=True,
        num_devices=NCORES,
    )
    AF = mybir.ActivationFunctionType

    xq_t = nc.dram_tensor("xq_t", [E, T], BF16, kind="ExternalInput").ap()
    xk_t = nc.dram_tensor("xk_t", [E, T], BF16, kind="ExternalInput").ap()
    xv_t = nc.dram_tensor("xv_t", [E, T], BF16, kind="ExternalInput").ap()
    wq_t = nc.dram_tensor("wq_t", [E, EH], BF16, kind="ExternalInput").ap()
    wk_t = nc.dram_tensor("wk_t", [E, EH], BF16, kind="ExternalInput").ap()
    wv_t = nc.dram_tensor("wv_t", [E, EH], BF16, kind="ExternalInput").ap()
    wo_t = nc.dram_tensor("wo_t", [EH, E], BF16, kind="ExternalInput").ap()
    bq_d = nc.dram_tensor("bq_d", [P, KHD], F32, kind="ExternalInput").ap()
    bk_d = nc.dram_tensor("bk_d", [P, KHD], F32, kind="ExternalInput").ap()
    bv_d = nc.dram_tensor("bv_d", [P, EH], F32, kind="ExternalInput").ap()
    mask_d = nc.dram_tensor("mask_d", [P, 4, TC], BF16, kind="ExternalInput").ap()
    out_p = nc.dram_tensor("out_part", [T, E], F32, kind="ExternalOutput").ap()

    from contextlib import ExitStack

    with tile.TileContext(nc) as tc, ExitStack() as ctx:
        persist = ctx.enter_context(tc.tile_pool(name="persist", bufs=1))
        xpool = ctx.enter_context(tc.tile_pool(name="x", bufs=2))
        exps = ctx.enter_context(tc.tile_pool(name="exps", bufs=8))
        evac = ctx.enter_context(tc.tile_pool(name="evac", bufs=3))

        # ---- constants / weights (scalar-engine HWDGE queue; x loads use sync)
        nc.gpsimd.load_library(library_config.attn)  # for partition_broadcast
        wq_sb = persist.tile([P, KO, EH], BF16, tag="wq")
        wk_sb = persist.tile([P, KO, EH], BF16, tag="wk")
        wv_sb = persist.tile([P, KO, EH], BF16, tag="wv")
        for w_sb, w_d in ((wq_sb, wq_t), (wk_sb, wk_t)):
            w_src = w_d.rearrange("(ko p) m -> p ko m", p=P)
            for ko in range(KO):
                nc.scalar.dma_start(w_sb[:, ko, :], w_src[:, ko, :])
        bq_sb = persist.tile([P, KHD], F32, tag="bq")
        nc.scalar.dma_start(bq_sb[:], bq_d)
        bk_sb = persist.tile([P, KHD], F32, tag="bk")
        nc.scalar.dma_start(bk_sb[:], bk_d)
        bv_sb = persist.tile([P, EH], F32, tag="bv")
        mask_sb = persist.tile([P, 4, TC], BF16, tag="mask")
        wo_sb = persist.tile([P, KHD, E], BF16, tag="wo")

        def emit_late_const_dmas():
            # wv/bv/mask/wo are needed only after the QK projections; emit
            # their DMAs behind the xq/xk chunks so startup isn't starved.
            wv_src = wv_t.rearrange("(ko p) m -> p ko m", p=P)
            for ko in range(KO):
                nc.scalar.dma_start(wv_sb[:, ko, :], wv_src[:, ko, :])
            nc.scalar.dma_start(bv_sb[:], bv_d)
            nc.scalar.dma_start(mask_sb[:], mask_d)
            wo_src = wo_t.rearrange("(ko p) m -> p ko m", p=P)
            for ko in range(KHD):
                nc.scalar.dma_start(wo_sb[:, ko, :], wo_src[:, ko, :])

        qt_sb = persist.tile([P, KHD, T], BF16, tag="qt")
        kt_sb = persist.tile([P, KHD, T], BF16, tag="kt")
        v_sb = persist.tile([P, NSB, HC * VW], BF16, tag="v")
        attnT = persist.tile([P, KHD, T], BF16, tag="attnT")

        # ---- phase 1: projections
        with tc.tile_pool(name="psA", bufs=2, space="PSUM") as psA:
            for x_dram, w_sb, b_sb, dst in (
                (xq_t, wq_sb, bq_sb, qt_sb),
                (xk_t, wk_sb, bk_sb, kt_sb),
            ):
                x_sb = xpool.tile([P, KO, T], BF16, tag="xt")
                x_src = x_dram.rearrange("(ko p) t -> p ko t", p=P)
                for ko in range(KO):
                    eng = nc.sync if ko % 2 == 0 else nc.scalar
                    eng.dma_start(x_sb[:, ko, :], x_src[:, ko, :])
                for db in range(KHD):
                    for tj in range(NT):
                        ps = psA.tile([P, TC], F32, tag="pp")
                        for ko in range(KO):
                            nc.tensor.matmul(
                                ps[:],
                                lhsT=w_sb[:, ko, db * P : (db + 1) * P],
                                rhs=x_sb[:, ko, tj * TC : (tj + 1) * TC],
                                start=(ko == 0),
                                stop=(ko == KO - 1),
                            )
                        nc.vector.tensor_scalar_add(
                            dst[:, db, tj * TC : (tj + 1) * TC],
                            ps[:],
                            b_sb[:, db : db + 1],
                        )

            emit_late_const_dmas()
            # V projection: out tiles [s-block 128, d 512], + bias, + ones cols
            xv_sb = xpool.tile([P, KO, T], BF16, tag="xt")
            xv_src = xv_t.rearrange("(ko p) t -> p ko t", p=P)
            for ko in range(KO):
                eng = nc.sync if ko % 2 == 0 else nc.scalar
                eng.dma_start(xv_sb[:, ko, :], xv_src[:, ko, :])
            for h in range(HC):
                nc.vector.memset(v_sb[:, :, h * VW + D : h * VW + VW], 1.0)
            for sb in range(NSB):
                ps = psA.tile([P, EH], F32, tag="pp")
                for ko in range(KO):
                    nc.tensor.matmul(
                        ps[:],
                        lhsT=xv_sb[:, ko, sb * P : (sb + 1) * P],
                        rhs=wv_sb[:, ko, :],
                        start=(ko == 0),
                        stop=(ko == KO - 1),
                    )
                v_dst = v_sb[:, sb, :].rearrange("p (h x) -> p h x", h=HC)[:, :, 0:D]
                nc.vector.tensor_add(
                    v_dst,
                    ps[:].rearrange("p (h x) -> p h x", h=HC),
                    bv_sb[:].rearrange("p (h x) -> p h x", h=HC),
                )

        # ---- phase 2+3 fused: attention with out-projection interleaved as
        # PE gap-fillers (keeps TensorE saturated while ACT computes exps,
        # avoiding HAM re-throttling and hiding the output projection).
        with (
            tc.tile_pool(name="psS", bufs=2, space="PSUM") as psS,
            tc.tile_pool(name="psPV", bufs=2, space="PSUM") as psPV,
            tc.tile_pool(name="psC", bufs=2, space="PSUM") as psC,
        ):
            fillers = []  # deferred out-proj tile emitters, popped into PE gaps

            def emit_filler():
                if fillers:
                    fillers.pop(0)()

            def make_outproj(tb, fj):
                def _emit():
                    po = psC.tile([P, TC], F32, tag="po")
                    for ko in range(KHD):
                        nc.tensor.matmul(
                            po[:],
                            lhsT=attnT[:, ko, tb * P : (tb + 1) * P],
                            rhs=wo_sb[:, ko, fj * TC : (fj + 1) * TC],
                            start=(ko == 0),
                            stop=(ko == KHD - 1),
                        )
                    ot = evac.tile([P, TC], F32, tag="ot")
                    nc.vector.tensor_copy(ot[:], po[:])
                    nc.gpsimd.dma_start(
                        out_p[tb * P : (tb + 1) * P, fj * TC : (fj + 1) * TC], ot[:]
                    )

                return _emit

            def make_norm(pv, pb, ch, tj):
                def _emit():
                    rec = evac.tile([1, TC], F32, tag="rec")
                    nc.vector.reciprocal(rec[:], pv[D : D + 1, :])
                    rbs = evac.tile([D, TC], F32, tag="rbs")
                    nc.gpsimd.partition_broadcast(rbs[:], rec[:])
                    nc.vector.tensor_mul(
                        attnT[pb : pb + D, ch, tj * TC : (tj + 1) * TC],
                        pv[0:D, :],
                        rbs[:],
                    )

                return _emit

            for tj in range(NT):
                slot = [0]
                for h in range(HC):
                    pb = D * (h % 2)
                    ch = h // 2
                    ng = 2 * tj + 2  # groups of 2 s-chunks
                    pv = psPV.tile([P, TC], F32, tag="pv")
                    ets = {}

                    def emit_pv(g, h=h, tj=tj, pv=pv):
                        et = ets.pop(g)
                        for u in range(2):
                            si = 2 * g + u
                            nc.tensor.matmul(
                                pv[0 : D + 1, :],
                                lhsT=v_sb[:, si, h * VW : (h + 1) * VW],
                                rhs=et[:, u, :],
                                start=(si == 0),
                                stop=(si == 4 * tj + 3),
                            )

                    for g in range(ng):
                        sc = psS.tile([P, 2, TC], F32, tag="sc")
                        for u in range(2):
                            si = 2 * g + u
                            nc.tensor.matmul(
                                sc[:, u, :],
                                lhsT=kt_sb[pb : pb + D, ch, si * P : (si + 1) * P],
                                rhs=qt_sb[pb : pb + D, ch, tj * TC : (tj + 1) * TC],
                                start=True,
                                stop=True,
                            )
                        et = exps.tile([P, 2, TC], BF16, tag="et")
                        if g >= 2 * tj:  # diagonal groups: partial-span exp
                            for u in range(2):
                                si = 2 * g + u
                                k = si - 4 * tj
                                if k > 0:
                                    nc.gpsimd.memset(et[:, u, 0 : P * k], 0.0)
                                nc.scalar.activation(
                                    et[:, u, P * k : TC],
                                    sc[:, u, P * k : TC],
                                    AF.Exp,
                                    scale=1.0 / math.sqrt(D),
                                )
                                nc.vector.tensor_mul(
                                    et[:, u, P * k : P * (k + 1)],
                                    et[:, u, P * k : P * (k + 1)],
                                    mask_sb[:, k, P * k : P * (k + 1)],
                                )
                        else:
                            nc.scalar.activation(
                                et[:], sc[:], AF.Exp, scale=1.0 / math.sqrt(D)
                            )
                        ets[g] = et
                        if g >= 2:
                            emit_pv(g - 2)  # PV runs two groups behind scores
                            slot[0] += 1
                            if h >= 2 and slot[0] % 2 == 0:
                                emit_filler()
                    for gg in range(max(0, ng - 2), ng):
                        emit_pv(gg)
                    make_norm(pv, pb, ch, tj)()
                # queue this tj's out-proj tiles; they fill PE gaps in tj+1
                for tb in range(4 * tj, 4 * tj + 4):
                    for fj in range(E // TC):
                        fillers.append(make_outproj(tb, fj))
            while fillers:
                emit_filler()

    nc.compile()
    return nc


def _get_nc():
    if "nc" not in _CACHE:
        _CACHE["nc"] = _build_nc()
    return _CACHE["nc"]


def _prep_in_maps(query, key, value, attn_mask, Wq, bq, Wk, bk, Wv, bv, Wo, bo):
    """Host-side prep: slices, transposes, bf16 casts. Returns in_maps[8]."""
    f32 = np.float32
    xt = {}  # (kind, b) -> [E, T] bf16
    for b in range(B):
        xt[("q", b)] = np.ascontiguousarray(query[:, b, :].T).astype(NPBF16)
        xt[("k", b)] = np.ascontiguousarray(key[:, b, :].T).astype(NPBF16)
        xt[("v", b)] = np.ascontiguousarray(value[:, b, :].T).astype(NPBF16)
    wt = {}
    for hg in range(2):
        sl = slice(EH * hg, EH * hg + EH)
        wt[("q", hg)] = np.ascontiguousarray(Wq[sl, :].T).astype(NPBF16)
        wt[("k", hg)] = np.ascontiguousarray(Wk[sl, :].T).astype(NPBF16)
        wt[("v", hg)] = np.ascontiguousarray(Wv[sl, :].T).astype(NPBF16)
        wt[("o", hg)] = np.ascontiguousarray(Wo[:, sl].T).astype(NPBF16)
        wt[("bq", hg)] = np.ascontiguousarray(
            bq[sl].astype(f32).reshape(KHD, P).T
        )
        wt[("bk", hg)] = np.ascontiguousarray(
            bk[sl].astype(f32).reshape(KHD, P).T
        )
        wt[("bv", hg)] = np.ascontiguousarray(
            np.tile(bv[sl].astype(f32)[None, :], (P, 1))
        )
    # mask patterns: for a scores tile with s0 = t0 + 128*o, pattern
    # [p, o, f] = 0 if attn_mask[t0+f, s0+p] (masked) else 1.
    t0 = 512
    patts = []
    for o in range(4):
        s0 = t0 + P * o
        patts.append(
            (~np.asarray(attn_mask[t0 : t0 + TC, s0 : s0 + P])).T.astype(NPBF16)
        )
    mask_tiles = np.ascontiguousarray(np.stack(patts, axis=1))  # [P, 4, TC]

    in_maps = []
    for c in range(NCORES):
        b, hg = c // 2, c % 2
        in_maps.append(
            {
                "xq_t": xt[("q", b)],
                "xk_t": xt[("k", b)],
                "xv_t": xt[("v", b)],
                "wq_t": wt[("q", hg)],
                "wk_t": wt[("k", hg)],
                "wv_t": wt[("v", hg)],
                "wo_t": wt[("o", hg)],
                "bq_d": wt[("bq", hg)],
                "bk_d": wt[("bk", hg)],
                "bv_d": wt[("bv", hg)],
                "mask_d": mask_tiles,
            }
        )
    return in_maps


def _run_on_hw(in_maps, trace=False, **kwargs):
    nc = _get_nc()
    return bass_utils.run_bass_kernel_spmd(
        nc, in_maps, core_ids=list(range(NCORES)), trace=trace, **kwargs
    )


def _gather(results, bo):
    outs = []
    for b in range(B):
        part = results[2 * b]["out_part"] + results[2 * b + 1]["out_part"]
        outs.append(part)
    out = np.stack(outs, axis=1)  # [T, B, E]
    out += np.asarray(bo, dtype=np.float32)[None, None, :]
    return out.astype(np.float32)


def _numpy_fallback(query, key, value, attn_mask, Wq, bq, Wk, bk, Wv, bv, Wo, bo):
    """Exact f32 numpy replication of the reference (for non-causal masks)."""
    f32 = np.float32
    query, key, value = (np.asarray(a, f32) for a in (query, key, value))
    q = (np.einsum("tbe,fe->btf", query, Wq, dtype=f32) + bq).reshape(B, T, H, D)
    k = (np.einsum("sbe,fe->bsf", key, Wk, dtype=f32) + bk).reshape(B, S, H, D)
    v = (np.einsum("sbe,fe->bsf", value, Wv, dtype=f32) + bv).reshape(B, S, H, D)
    q, k, v = (a.transpose(0, 2, 1, 3) for a in (q, k, v))
    out = np.empty((B, H, T, D), f32)
    mask = np.asarray(attn_mask)
    for b in range(B):
        for h in range(H):
            sc = (q[b, h] @ k[b, h].T) / np.float32(math.sqrt(D))
            sc = np.where(mask, -np.inf, sc)
            m = np.max(sc, axis=-1, keepdims=True)
            m = np.where(np.isfinite(m), m, 0.0)
            e = np.exp(sc - m)
            p = e / np.sum(e, axis=-1, keepdims=True)
            p = np.where(np.isinf(sc), 0.0, p)
            out[b, h] = p @ v[b, h]
    out = out.transpose(0, 2, 1, 3).reshape(B, T, E)
    out = out @ np.asarray(Wo, f32).T + bo
    return np.ascontiguousarray(out.transpose(1, 0, 2)).astype(f32)


def kernel(query, key, value, attn_mask, Wq, bq, Wk, bk, Wv, bv, Wo, bo):
    mask = np.asarray(attn_mask)
    causal = mask.shape == (T, S) and np.array_equal(
        mask, np.triu(np.ones((T, S), dtype=bool), k=1)
    )
    if not causal:
        return _numpy_fallback(
            query, key, value, attn_mask, Wq, bq, Wk, bk, Wv, bv, Wo, bo
        )
    in_maps = _prep_in_maps(
        query, key, value, attn_mask, Wq, bq, Wk, bk, Wv, bv, Wo, bo
    )
    res = _run_on_hw(in_maps)
    return _gather(res.results, bo)

